# revision 21
# baseline (speedup 1.0000x reference)
"""BiLSTM-CRF Trainium2 kernel (8-core data-parallel over batch).

Contract: kernel(**inputs) takes the FULL unsharded inputs (numpy arrays,
keyed as in reference.setup_inputs()) and returns the FULL [B, T, TAGS, TAGS]
crf_scores array. All model compute runs on the 8 NeuronCores; the host only
does weight re-layout / dtype packing and input sharding.
"""
import sys
import types
from contextlib import ExitStack

import ml_dtypes
import numpy as np

import concourse.bacc as bacc
import concourse.bass as bass
import concourse.mybir as mybir
import concourse.tile as tile
from concourse import library_config
from concourse.bass_utils import run_bass_kernel_spmd

# ---- problem dims (hardcoded per spec) ----
VOCAB = 30000
EMB = 256
HD = 128          # per-direction hidden
G4 = 512          # 4*HD gates
TAGS = 16
B, T = 64, 512
NCORES = 8
BC = B // NCORES  # batch per core = 8
NT = BC * T       # tokens per core = 4096

BF16 = mybir.dt.bfloat16
F32 = mybir.dt.float32
I16 = mybir.dt.int16
AF = mybir.ActivationFunctionType
ALU = mybir.AluOpType

# gate order in reference (jnp.split): i, f, g, o. We reorder to [f, i, o, g]
# so one sigmoid op covers blocks [f|i|o] and the g block rides along as
# sigma(2x) (tanh(x) = 2*sigma(2x) - 1, g-weights doubled on host).
_PERM = np.concatenate([
    np.arange(128, 256),   # f
    np.arange(0, 128),     # i
    np.arange(384, 512),   # o
    np.arange(256, 384),   # g
])


def _ensure_ntff_hook():
    """The RL image's antenv lacks axon_hooks; inject it so trace=True works."""
    if "antenv.axon_hooks" in sys.modules:
        return
    mod = types.ModuleType("antenv.axon_hooks")
    mod._hook = None
    mod.set_axon_ntff_profile_hook = lambda h: setattr(mod, "_hook", h)
    mod.get_axon_ntff_profile_hook = lambda: mod._hook
    sys.modules["antenv.axon_hooks"] = mod
    try:
        import antenv
        antenv.axon_hooks = mod
        from trn_agent_boot.trn_boot import _ntff_profile_via_ctypes
        mod.set_axon_ntff_profile_hook(
            _ntff_profile_via_ctypes("/opt/axon/libaxon_pjrt.so"))
    except Exception:
        pass


def build(t_steps: int = T, bc: int = BC):
    """Build + compile the per-core Bass program. t_steps/bc shrinkable for sim."""
    nt = bc * t_steps
    assert nt % 512 == 0
    nc = bacc.Bacc("TRN2", target_bir_lowering=False, debug=False,
                   num_swdge_queues=4)

    # ---- DRAM I/O ----
    emb_d = nc.dram_tensor("emb", [VOCAB, EMB], BF16, kind="ExternalInput")
    idx_d = nc.dram_tensor("idx", [128, nt // 16], I16, kind="ExternalInput")
    wihT_d = {d: nc.dram_tensor(f"wihT_{d}", [EMB, G4], BF16, kind="ExternalInput")
              for d in "fb"}
    whhT_d = {d: nc.dram_tensor(f"whhT_{d}", [HD, G4], BF16, kind="ExternalInput")
              for d in "fb"}
    bias_d = {d: nc.dram_tensor(f"bias_{d}", [128, 4], F32, kind="ExternalInput")
              for d in "fb"}
    woutT_d = nc.dram_tensor("woutT", [2, HD, TAGS], BF16, kind="ExternalInput")
    trans_d = nc.dram_tensor("trans", [128, TAGS * TAGS], F32, kind="ExternalInput")
    ident_d = nc.dram_tensor("ident", [128, 128], BF16, kind="ExternalInput")
    crf_d = nc.dram_tensor("crf", [nt, TAGS * TAGS], F32, kind="ExternalOutput")

    NB = nt // 512   # 512-token gather/gemm blocks
    TPB = 512 // bc  # timesteps per 512-token block

    with tile.TileContext(nc) as tc, ExitStack() as ctx:
        const = ctx.enter_context(tc.tile_pool(name="const", bufs=1))
        big = ctx.enter_context(tc.tile_pool(name="big", bufs=1))

        # ---- persistent SBUF ----
        idx_sb = const.tile([128, nt // 16], I16)
        wihT = {d: const.tile([128, 2, G4], BF16, tag=f"wihT{d}", name=f"wihT{d}") for d in "fb"}
        whhT = {d: const.tile([HD, G4], BF16, tag=f"whhT{d}", name=f"whhT{d}") for d in "fb"}
        bias = {d: const.tile([128, 4], F32, tag=f"bias{d}", name=f"bias{d}") for d in "fb"}
        woutT = const.tile([HD, 2, TAGS], BF16)
        trans = const.tile([128, TAGS * TAGS], F32)
        ident = const.tile([128, 128], BF16)

        xT = big.tile([128, NB, 2, 512], BF16, tag="xT")
        zin = {d: big.tile([128, t_steps, 4, bc], BF16, tag=f"zin{d}", name=f"zin{d}") for d in "fb"}
        # h histories: h_f[:, t+1, :] = fwd hidden at time t (col 0 = zeros)
        #              h_b[:, u, :]   = bwd hidden at time u (col t_steps = zeros)
        # Split into 16-col segment tiles aligned to the emission chunks'
        # reads so each chunk's dependency resolves as soon as its 16
        # timesteps exist (readiness spreads over scan steps ~272..512
        # instead of every chunk waiting for scan end).
        HSEG = 16
        assert t_steps % HSEG == 0
        hh = {}
        for d in "fb":
            segs = ([(0, 1)] + [(1 + HSEG * k, HSEG) for k in range(t_steps // HSEG)]
                    if d == "f" else
                    [(HSEG * k, HSEG) for k in range(t_steps // HSEG)] + [(t_steps, 1)])
            hh[d] = [(s0, n, big.tile([128, n, bc], BF16, tag=f"hh{d}{s0}",
                                      name=f"hh{d}{s0}")) for s0, n in segs]

        def hcol(d, col):
            for s0, n, tl in hh[d]:
                if s0 <= col < s0 + n:
                    return tl[:, col - s0, :]
            raise AssertionError((d, col))

        def hspan(d, c0, n):
            """[128, n, bc] slice over cols [c0, c0+n); must be one segment."""
            for s0, sn, tl in hh[d]:
                if s0 == c0 and sn == n:
                    return tl[:, 0:n, :]
            raise AssertionError((d, c0, n))
        C2 = {d: const.tile([128, 2 * bc], F32, tag=f"C2{d}", name=f"C2{d}") for d in "fb"}

        # ---- load inputs ----
        # Only idx goes on the sync queue up front: the gpsimd library-load's
        # payload shares that queue, and stacking all 10 weight DMAs first
        # delays the embedding gathers (and so scan start) by ~6us.
        nc.sync.dma_start(idx_sb[:], idx_d[:])
        for d in "fb":
            nc.vector.memset(C2[d][:], 0.0)
        nc.gpsimd.memset(hcol("f", 0), 0.0)
        nc.gpsimd.memset(hcol("b", t_steps), 0.0)

        # ---- embedding gather straight into x.T layout ----
        # block order: fwd scan consumes blocks ascending, bwd descending, so
        # produce [0, NB-1, 1, NB-2, ...] to unblock both scan heads ASAP.
        border = []
        for k in range((NB + 1) // 2):
            border.append(k)
            if NB - 1 - k != k:
                border.append(NB - 1 - k)
        nc.gpsimd.load_library(library_config.mlp)
        for qi, nb in enumerate(border):
            # spread gathers across SWDGE queues so they execute in parallel
            # (serialized on one queue they finish 4.5us apart, stalling the
            # early scan's interleaved projections on the in-order PE)
            nc.gpsimd.dma_gather(
                xT[:, nb, :, :],
                emb_d[:, :],
                idx_sb[:, 32 * nb:32 * (nb + 1)],
                512, 512, EMB,
                transpose=True,
                queue_num=qi % 4,
            )

        # ---- weights / constants (after the gathers are queued) ----
        for d in "fb":
            nc.sync.dma_start(wihT[d][:], wihT_d[d].rearrange("(k p) g -> p k g", p=128))
            nc.sync.dma_start(whhT[d][:], whhT_d[d][:])
            nc.sync.dma_start(bias[d][:], bias_d[d][:])
        nc.sync.dma_start(woutT[:], woutT_d.rearrange("c h t -> h c t"))
        nc.sync.dma_start(trans[:], trans_d[:])
        nc.sync.dma_start(ident[:], ident_d[:])

        # ---- input projections: zin = x @ Wih.T + b ----
        # Only the two scan-head blocks -- (0, fwd) and (NB-1, bwd) -- are
        # projected up front; the in-order PE queue would otherwise delay the
        # scan's first gate matmul behind all 128 projection matmuls (~40us).
        # The remaining (block, dir) pairs are interleaved into the scan loop
        # in consumption-deadline order, one (block, dir, gate-quarter) group
        # every few steps; each fits inside the scan's per-step PE idle gap.
        # zpsum stays open through the scan so the scan psum pool lands on
        # different banks (no pool-release serialization).
        zpsum = ctx.enter_context(tc.tile_pool(name="zpsum", bufs=2, space="PSUM"))

        def proj_group(nb, d, c, copy_engine):
            zp = zpsum.tile([128, 512], F32, tag="zp")
            nc.tensor.matmul(
                zp[:], wihT[d][:, 0, 128 * c:128 * (c + 1)],
                xT[:, nb, 0, :],
                start=True, stop=False)
            nc.tensor.matmul(
                zp[:], wihT[d][:, 1, 128 * c:128 * (c + 1)],
                xT[:, nb, 1, :],
                start=False, stop=True)
            # strided copyback into [t, c, b] layout, bias folded in
            dst = zin[d][:, TPB * nb:TPB * (nb + 1), c, :]
            if copy_engine == "act":
                nc.scalar.activation(dst, zp[:], AF.Identity,
                                     bias=bias[d][:, c:c + 1])
            else:
                # One half on DVE, one on ACT: the scheduler front-loads all
                # deferred copybacks into the early scan steps, so balance
                # the crunch across both elementwise engines.
                half = TPB // 2
                nc.vector.tensor_scalar_add(
                    zin[d][:, TPB * nb:TPB * nb + half, c, :],
                    zp[:, 0:256], bias[d][:, c:c + 1])
                nc.scalar.activation(
                    zin[d][:, TPB * nb + half:TPB * (nb + 1), c, :],
                    zp[:, 256:512], AF.Identity, bias=bias[d][:, c:c + 1])

        for nb, d in ((0, "f"), (NB - 1, "b")):
            for c in range(4):
                proj_group(nb, d, c, "act")

        # deadline order for the deferred pairs: fwd block k needed at step
        # 64*k, bwd block k needed at step T-64*(k+1).
        deferred = []
        for k in range(1, NB):
            deferred.append((k, "f"))
            deferred.append((NB - 1 - k, "b"))
        deferred_groups = [(nb, d, c) for nb, d in deferred for c in range(4)]
        PROJ_START = 24   # first scan step that emits a deferred group
        PROJ_EVERY = 4    # steps between deferred groups (deadline-checked)

        # ---- the recurrent scan (fwd + bwd interleaved) ----
        # Per-direction critical chain per step:
        #   PE gates -> ACT sigmoid -> DVE Pi2 -> DVE c' -> ACT tanh -> DVE h
        # The sf*c product (Pf) runs OFF-chain on gpsimd in parallel with Pi2
        # (on DVE for the first GP_START steps while gpsimd still owns the
        # embedding gathers). Algebra: si*tanh(zg) = 2*(sigma(2 zg)-0.5)*si,
        # so  c' = (Pi2 * 2) + Pf  with  Pi2 = (S_g - 0.5)*S_i.
        GP_START = 10 ** 9  # Pf always on DVE (measured faster than gpsimd)
        # All pools (scan + emission) enter the same ExitStack: a pool opened
        # after another closes would alias its PSUM banks, and the release
        # dependency would pin every emission op past the end of the scan.
        with tc.tile_pool(name="spsum", bufs=4, space="PSUM") as spsum, \
             tc.tile_pool(name="sS", bufs=4) as sS, \
             tc.tile_pool(name="sP", bufs=4) as sP, \
             tc.tile_pool(name="sF", bufs=4) as sF, \
             tc.tile_pool(name="sC", bufs=4) as sC, \
             tc.tile_pool(name="sT", bufs=4) as sT, \
             tc.tile_pool(name="epsum", bufs=2, space="PSUM") as epsum, \
             tc.tile_pool(name="ecrf", bufs=4) as ecrf:

            def new_z(t):
                """Fresh psum tiles for step t with zin injected (identity mm).
                Emitted one step ahead so gate mms fire as soon as h lands."""
                u = t_steps - 1 - t
                zt = {}
                for d, time in (("f", t), ("b", u)):
                    zt[d] = spsum.tile([128, 4 * bc], F32, tag="z", name=f"z{d}")
                    nc.tensor.matmul(zt[d][:], ident[:],
                                     zin[d][:, time, :, :],
                                     start=True, stop=False)
                return zt

            c_prev = {d: C2[d][:, 0:bc] for d in "fb"}  # zeroed in prologue
            z = new_z(0)
            for t in range(t_steps):
                if (t >= PROJ_START and (t - PROJ_START) % PROJ_EVERY == 0
                        and deferred_groups):
                    proj_group(*deferred_groups.pop(0), "dve")
                u = t_steps - 1 - t  # bwd time index
                for d, rd_col in (("f", t), ("b", u + 1)):
                    for c in range(4):
                        nc.tensor.matmul(
                            z[d][:, bc * c:bc * (c + 1)],
                            whhT[d][:, 128 * c:128 * (c + 1)],
                            hcol(d, rd_col),
                            start=False, stop=(c == 3))
                z_cur, z = z, (new_z(t + 1) if t + 1 < t_steps else None)
                for d, wr_col in (("f", t + 1), ("b", u)):
                    S = sS.tile([128, 4 * bc], F32, tag="S")
                    nc.scalar.activation(S[:], z_cur[d][:], AF.Sigmoid)
                    # Pf = sf * c_{t-1}: off-chain (gpsimd once gathers drain)
                    Pf = sF.tile([128, bc], F32, tag="Pf")
                    eng = nc.gpsimd if t >= GP_START else nc.vector
                    eng.tensor_tensor(Pf[:], S[:, 0:bc], c_prev[d], ALU.mult)
                    # Pi2 = (sigma(2 zg) - 0.5) * si  (= si*gtilde/2), on-chain
                    Pi2 = sP.tile([128, bc], F32, tag="Pi2")
                    nc.vector.scalar_tensor_tensor(
                        Pi2[:], S[:, 3 * bc:4 * bc], 0.5, S[:, bc:2 * bc],
                        ALU.subtract, ALU.mult)
                    cn = sC.tile([128, bc], F32, tag="c")
                    nc.vector.scalar_tensor_tensor(
                        cn[:], Pi2[:], 2.0, Pf[:], ALU.mult, ALU.add)
                    c_prev[d] = cn[:]
                    TC = sT.tile([128, bc], F32, tag="TC")
                    nc.scalar.activation(TC[:], cn[:], AF.Tanh)
                    nc.vector.tensor_tensor(hcol(d, wr_col),
                                            S[:, 2 * bc:3 * bc], TC[:], ALU.mult)

            # ---- emission + CRF broadcast-add + store ----
            TOK = 128        # tokens per output chunk
            tpc = TOK // bc  # timesteps per chunk
            nchunks = nt // TOK
            # emit chunks in scan-readiness order (middle-out): chunk n is
            # ready at scan step max(16n+16, 512-16n)
            order = sorted(range(nchunks),
                           key=lambda n: max(tpc * n + tpc, t_steps - tpc * n))
            for nb in order:
                t0 = nb * tpc
                e = epsum.tile([128, TAGS], F32, tag="e")
                nc.tensor.matmul(e[:], hspan("f", t0 + 1, tpc),
                                 woutT[:, 0, :], start=True, stop=False)
                nc.tensor.matmul(e[:], hspan("b", t0, tpc),
                                 woutT[:, 1, :], start=False, stop=True)
                crf_sb = ecrf.tile([128, TAGS * TAGS], F32, tag="crf")
                e_b = e[:, None, :].to_broadcast([128, TAGS, TAGS])
                nc.vector.tensor_tensor(crf_sb[:], e_b, trans[:], ALU.add)
                nc.sync.dma_start(crf_d[TOK * nb:TOK * (nb + 1), :], crf_sb[:])

    nc.compile()
    _assert_ldw_pairing(nc)
    return nc


def _assert_ldw_pairing(nc):
    """Every non-self-loading matmul must directly follow an InstLdweights
    whose weights AP matches the matmul's weights operand. CoreSim ignores
    InstLdweights, so a pairing break would only show up as wrong results on
    hardware — catch it at build time instead."""
    for f in nc.m.functions:
        for bb in f.blocks:
            prev_pe = None
            for ins in bb.instructions:
                if ins.engine != mybir.EngineType.PE:
                    continue
                if isinstance(ins, mybir.InstMatmult) and ins.ldweights is False:
                    assert isinstance(prev_pe, mybir.InstLdweights), (
                        f"{ins.name}: non-self-loading matmul not preceded by "
                        f"ldweights (got {type(prev_pe).__name__})")
                    assert repr(prev_pe.ins[0]) == repr(ins.ins[1]), (
                        f"{ins.name}: weights mismatch with {prev_pe.name}")
                prev_pe = ins


_CACHE = {}


def _get_nc():
    if "nc" not in _CACHE:
        _CACHE["nc"] = build()
    return _CACHE["nc"]


def _prep_dir(w_ih, w_hh, b):
    w_ih = np.asarray(w_ih, np.float32)[_PERM].copy()
    w_hh = np.asarray(w_hh, np.float32)[_PERM].copy()
    b = np.asarray(b, np.float32)[_PERM].copy()
    w_ih[384:512] *= 2.0
    w_hh[384:512] *= 2.0
    b[384:512] *= 2.0
    wihT = np.ascontiguousarray(w_ih.T).astype(ml_dtypes.bfloat16)
    whhT = np.ascontiguousarray(w_hh.T).astype(ml_dtypes.bfloat16)
    bias = np.ascontiguousarray(b.reshape(4, 128).T).astype(np.float32)
    return wihT, whhT, bias


def make_in_maps(sentences, embedding, W_ih_f, W_hh_f, b_f, W_ih_b, W_hh_b,
                 b_b, W_out, b_out, transition):
    emb = np.asarray(embedding, np.float32).astype(ml_dtypes.bfloat16)
    wihT_f, whhT_f, bias_f = _prep_dir(W_ih_f, W_hh_f, b_f)
    wihT_b, whhT_b, bias_b = _prep_dir(W_ih_b, W_hh_b, b_b)
    wo = np.asarray(W_out, np.float32)  # [16, 256]
    woutT = np.stack([np.ascontiguousarray(wo[:, :128].T),
                      np.ascontiguousarray(wo[:, 128:].T)])
    woutT = woutT.astype(ml_dtypes.bfloat16)  # [2, 128, 16]
    trans_aug = (np.asarray(transition, np.float32)
                 + np.asarray(b_out, np.float32)[None, :]).reshape(-1)  # [256]
    trans_rep = np.ascontiguousarray(
        np.broadcast_to(trans_aug, (128, 256))).astype(np.float32)
    ident = np.eye(128, dtype=ml_dtypes.bfloat16)

    sent = np.asarray(sentences).astype(np.int64)
    in_maps = []
    for c in range(NCORES):
        toks = sent[BC * c:BC * (c + 1)].T.reshape(-1)  # (t, b) order, [4096]
        idx = np.tile(toks.reshape(NT // 16, 16).T.astype(np.int16), (8, 1))
        in_maps.append({
            "emb": emb, "idx": idx,
            "wihT_f": wihT_f, "wihT_b": wihT_b,
            "whhT_f": whhT_f, "whhT_b": whhT_b,
            "bias_f": bias_f, "bias_b": bias_b,
            "woutT": woutT, "trans": trans_rep, "ident": ident,
        })
    return in_maps


def assemble_out(results):
    out = np.empty((B, T, TAGS, TAGS), np.float32)
    for c in range(NCORES):
        crf = results[c]["crf"].reshape(T, BC, TAGS, TAGS)
        out[BC * c:BC * (c + 1)] = crf.transpose(1, 0, 2, 3)
    return out


def kernel(**inputs):
    _ensure_ntff_hook()
    nc = _get_nc()
    in_maps = make_in_maps(**inputs)
    res = run_bass_kernel_spmd(nc, in_maps, list(range(NCORES)))
    return assemble_out(res.results)

